# revision 4
# baseline (speedup 1.0000x reference)
"""Trainium2 Bass kernel for attention-LSTM decoder (teacher-forced), v2.

Reference computation (per batch element b, S=21 steps):
    Hp = batch_H @ Wi.T                      [B,T,H]   (precomputed once)
    per step s:
        ph    = h @ Wh.T + bh                [B,H]
        e     = tanh(Hp + ph[:,None,:]) @ Ws [B,T]
        alpha = softmax(e, axis=T)
        ctx   = alpha @ batch_H              [B,D]
        gates = [ctx,oh] @ Wih.T + bih + h @ Whh.T + bhh
        LSTM pointwise -> h, c
    probs = hs @ Wg.T + bg                   [B,S,C]

Sharding: data-parallel over batch (1024 -> 128 per core x 8 cores),
weights replicated, recurrence local per core.

v2 changes vs baseline:
  - batch_H ([b,t,d] operand of the ctx matmul) fully RESIDENT in SBUF:
    removes the 8.4MB/step DMA stream that saturated the single HW queue.
  - score phase in 8 chunks of 8 t-steps; Z = Hp + ph via ONE 4D DVE op
    per chunk; tanh per chunk in-place; per-chunk exp/diag/ctx flush with
    one-chunk skew so PE/ACT/DVE pipeline.
  - cross-step tail tightened: ctx evict -> PE transpose -> gates-x
    pipelined per d-chunk; pointwise f-first; h transposes pipelined;
    ph evicted on ACT (DVE busy); probs bias folded into the DVE evict
    (PRB = bg replicated host-side) instead of a PE matmul.
  - preamble DMA split across both HWDGE queues (sync + scalar), Hp PSUM
    eviction alternates DVE/ACT.
"""

import numpy as np
import ml_dtypes

import sys

sys.path.insert(0, "/opt/trn_rl_repo")

import concourse.bass as bass  # noqa: E402
import concourse.mybir as mybir  # noqa: E402
import concourse.tile as tile  # noqa: E402
from concourse import bacc  # noqa: E402
from concourse.bass_utils import run_bass_kernel_spmd  # noqa: E402

BF16 = mybir.dt.bfloat16
F32 = mybir.dt.float32
AF = mybir.ActivationFunctionType
ALU = mybir.AluOpType

B, T, D, H, C, S = 1024, 64, 512, 512, 96, 21
NCORES = 8
BC = B // NCORES  # 128 batch per core
HK = H // 128  # 4 h chunks
DK = D // 128  # 4 d chunks
NTB = T * BC  # 8192 flattened (t,b), t-major
CHUNKS = [(0, 4)] + [(4 + 8 * i, 8) for i in range(7)] + [(60, 4)]
NCH = len(CHUNKS)  # 9 score chunks

_CACHE = {}


def _build():
    nc = bacc.Bacc(
        "TRN2",
        target_bir_lowering=False,
        debug=False,
        enable_asserts=False,
        num_devices=1,
    )

    # ---- DRAM I/O (per-core shapes) ----
    d_bht = nc.dram_tensor("bht", [D, T, BC], BF16, kind="ExternalInput").ap()
    d_bhres = nc.dram_tensor("bhres", [BC, T, D], BF16, kind="ExternalInput").ap()
    d_wit = nc.dram_tensor("wit", [DK, 128, H], BF16, kind="ExternalInput").ap()
    d_wcat = nc.dram_tensor("wcat", [9, 128, 4 * H], BF16, kind="ExternalInput").ap()
    d_wht = nc.dram_tensor("wht", [HK, 128, H], BF16, kind="ExternalInput").ap()
    d_wgt = nc.dram_tensor("wgt", [HK, 128, C], BF16, kind="ExternalInput").ap()
    d_wsp = nc.dram_tensor("wsp", [128, HK], BF16, kind="ExternalInput").ap()
    d_bhb = nc.dram_tensor("bhb", [128, HK], F32, kind="ExternalInput").ap()
    d_oht = nc.dram_tensor("oht", [128, S, BC], BF16, kind="ExternalInput").ap()
    d_prb = nc.dram_tensor("prb", [128, C], BF16, kind="ExternalInput").ap()
    d_idbf = nc.dram_tensor("idbf", [128, 128], BF16, kind="ExternalInput").ap()
    d_out = nc.dram_tensor("probs", [BC, S, C], F32, kind="ExternalOutput").ap()

    with tile.TileContext(nc) as tc:
        import contextlib

        es = contextlib.ExitStack()
        with es:
            singles = es.enter_context(tc.tile_pool(name="singles", bufs=1))

            # ---- resident tensors ----
            HPR = singles.tile([128, HK, NTB], BF16, tag="hpr")
            BHR = singles.tile([128, T, D], BF16, tag="bhr")
            WCAT = singles.tile([128, 9, 4 * H], BF16, tag="wcat")
            WHT = singles.tile([128, HK, H], BF16, tag="wht")
            WGT = singles.tile([128, HK, C], BF16, tag="wgt")
            WSP = singles.tile([128, HK], BF16, tag="wsp")
            BHB = singles.tile([128, HK], F32, tag="bhb")
            PRB = singles.tile([128, C], BF16, tag="prb")
            IDBF = singles.tile([128, 128], BF16, tag="idbf")
            ESB = singles.tile([BC, T], BF16, tag="esb")
            SUMS = singles.tile([BC, NCH], F32, tag="sums")
            RS = singles.tile([BC, 1], F32, tag="rs")
            CS = singles.tile([BC, H], BF16, tag="cstate")

            # small weights on sync queue up front
            nc.sync.dma_start(out=WSP, in_=d_wsp)
            nc.sync.dma_start(out=BHB, in_=d_bhb)
            nc.sync.dma_start(out=IDBF, in_=d_idbf)
            nc.sync.dma_start(out=PRB, in_=d_prb)
            for k in range(HK):
                nc.sync.dma_start(out=WHT[:, k, :], in_=d_wht[k])
                nc.sync.dma_start(out=WGT[:, k, :], in_=d_wgt[k])

            nc.vector.memset(CS, 0.0)

            # ---- preamble machinery (fused into step 0) ----
            # Hp = batch_H @ Wi.T (+bh) is computed INSIDE step 0''s score
            # phase: block nb covers t-span [4nb, 4nb+4); step-0 chunk c is
            # emitted as soon as its t-span of HPR is complete. Hp MM groups
            # accumulate in the gate PSUM tiles (free until step-0 gates).
            bhtp = es.enter_context(tc.tile_pool(name="bhtp", bufs=2))
            WIT = bhtp.tile([128, DK, H], BF16, tag="wit", bufs=1)
            for k in range(DK):
                nc.sync.dma_start(out=WIT[:, k, :], in_=d_wit[k])

            # ---- step-loop pools ----
            xpool = es.enter_context(tc.tile_pool(name="xpool", bufs=2))
            dpool = es.enter_context(tc.tile_pool(name="dpool", bufs=2))
            phpool = es.enter_context(tc.tile_pool(name="phpool", bufs=2))
            htpool = es.enter_context(tc.tile_pool(name="htpool", bufs=2))
            actp = es.enter_context(tc.tile_pool(name="actp", bufs=2))
            fpool = es.enter_context(tc.tile_pool(name="fpool", bufs=2))
            ctxp = es.enter_context(tc.tile_pool(name="ctxp", bufs=1))
            xtp = es.enter_context(tc.tile_pool(name="xtp", bufs=1))
            prp = es.enter_context(tc.tile_pool(name="prp", bufs=1))
            ohp = es.enter_context(tc.tile_pool(name="ohp", bufs=3))

            e_psp = es.enter_context(tc.tile_pool(name="e_ps", bufs=1, space="PSUM"))
            ctx_psp = es.enter_context(
                tc.tile_pool(name="ctx_ps", bufs=1, space="PSUM")
            )
            g_psp = es.enter_context(tc.tile_pool(name="g_ps", bufs=1, space="PSUM"))
            sm_psp = es.enter_context(tc.tile_pool(name="sm_ps", bufs=1, space="PSUM"))

            # one-hot stationary tiles, streamed 2 steps ahead
            oh_tiles = {}
            for s in (0, 1):
                oh_tiles[s] = ohp.tile([128, BC], BF16, tag="oh", name="oh")
                nc.sync.dma_start(out=oh_tiles[s], in_=d_oht[:, s, :])

            # initial ph = 0 (h0 = 0), initial hT = 0
            ph_sb = phpool.tile([128, HK, BC], BF16, tag="ph")
            nc.vector.memset(ph_sb, 0.0)
            hT = htpool.tile([128, HK, BC], BF16, tag="ht")
            nc.vector.memset(hT, 0.0)

            pre_nb = [0]
            pr_pending = [None]

            def emit_pr_evict():
                pr, s_idx = pr_pending[0]
                pr_pending[0] = None
                pr_sb = prp.tile([128, C], F32, tag="pr_sb", name="pr_sb")
                nc.vector.tensor_tensor(
                    out=pr_sb, in0=pr[:, 0:C], in1=PRB, op=ALU.add
                )
                nc.sync.dma_start(out=d_out[:, s_idx, :], in_=pr_sb)

            def emit_pre_block(nb):
                eng = nc.sync if nb % 2 == 0 else nc.scalar
                eng2 = nc.scalar if nb % 2 == 0 else nc.sync
                pss = [
                    g_psp.tile([128, 512], F32, tag=f"gate{mh}", name="hp_ps")
                    for mh in range(HK)
                ]
                for j in range(2):
                    bt = bhtp.tile([128, 2, 512], BF16, tag="bht_in", name="bt")
                    eng.dma_start(
                        out=bt,
                        in_=d_bht[
                            2 * j * 128 : (2 * j + 2) * 128, 4 * nb : 4 * nb + 4, :
                        ].rearrange("(u p) t b -> p u (t b)", u=2),
                    )
                    for u in range(2):
                        kd = 2 * j + u
                        for mh in range(HK):
                            nc.tensor.matmul(
                                pss[mh],
                                WIT[:, kd, mh * 128 : (mh + 1) * 128],
                                bt[:, u, :],
                                start=(kd == 0),
                                stop=(kd == DK - 1),
                            )
                for mh in range(HK):
                    dst = HPR[:, mh, nb * 512 : (nb + 1) * 512]
                    if mh % 2 == 0:
                        nc.scalar.activation(
                            out=dst,
                            in_=pss[mh],
                            func=AF.Identity,
                            bias=BHB[:, mh : mh + 1],
                        )
                    else:
                        nc.vector.tensor_scalar(
                            out=dst,
                            in0=pss[mh],
                            scalar1=BHB[:, mh : mh + 1],
                            scalar2=None,
                            op0=ALU.add,
                        )
                # resident batch_H piece rides the opposite queue
                eng2.dma_start(
                    out=BHR[:, 4 * nb : 4 * nb + 4, :],
                    in_=d_bhres[:, 4 * nb : 4 * nb + 4, :],
                )
                if nb == 15:
                    # fat gate weights after the streaming blocks; k=4
                    # (onehot/bias) first -- needed by step-0 gates
                    for k in (4, 0, 1, 2):
                        nc.sync.dma_start(out=WCAT[:, k, :], in_=d_wcat[k])
                    for k in (3, 5, 6, 7, 8):
                        nc.scalar.dma_start(out=WCAT[:, k, :], in_=d_wcat[k])

            NGO = [1, 0, 3, 2]  # emission order f,i,g,o (perm layout i,f,o,g)

            for s in range(S):
                if s + 2 < S:
                    oh_tiles[s + 2] = ohp.tile([128, BC], BF16, tag="oh", name="oh")
                    nc.sync.dma_start(out=oh_tiles[s + 2], in_=d_oht[:, s + 2, :])

                # -- gates h-part + onehot/bias part: only needs hT(s-1) --
                # one PSUM tile per gate so each activation depends only on
                # its own gate's matmuls (Tile tracks whole-tile deps).
                # At s==0 (h == 0) the h-part is skipped and the ohbias MM
                # is deferred to just before the x-part (WCAT arrives late).
                g_t = {}
                if s > 0:
                    for ng in NGO:
                        g_t[ng] = g_psp.tile(
                            [128, 512], F32, tag=f"gate{ng}", name="g_t"
                        )
                    for ng in NGO:
                        for j, k in enumerate((5, 6, 7, 8, 4)):
                            lhs = oh_tiles[s] if k == 4 else hT[:, k - 5, :]
                            nc.tensor.matmul(
                                g_t[ng],
                                lhs,
                                WCAT[:, k, ng * 512 : (ng + 1) * 512],
                                start=(j == 0),
                                stop=False,
                            )

                # -- score phase: 8 chunks x 8 t, one-chunk-skew pipeline --
                ctx_ps = ctx_psp.tile([128, D], F32, tag="ctx")
                pend = []  # deferred flush state: (c, e_ps)

                def flush(c, e_ps):
                    t0, tn = CHUNKS[c]
                    nc.scalar.activation(
                        out=ESB[:, t0 : t0 + tn],
                        in_=e_ps[:, 0:tn],
                        func=AF.Exp,
                        accum_out=SUMS[:, c : c + 1],
                    )
                    dg = dpool.tile([128, 8, 128], BF16, tag="diag", name="dg")
                    nc.vector.tensor_tensor(
                        out=dg[:, 0:tn, :],
                        in0=IDBF.unsqueeze(1).broadcast_to([128, tn, 128]),
                        in1=ESB[:, t0 : t0 + tn]
                        .unsqueeze(2)
                        .broadcast_to([128, tn, 128]),
                        op=ALU.mult,
                    )
                    for tl in range(tn):
                        t = t0 + tl
                        nc.tensor.matmul(
                            ctx_ps,
                            dg[:, tl, :],
                            BHR[:, t, :],
                            start=(t == 0),
                            stop=(t == T - 1),
                        )

                for c in range(NCH):
                    t0, tn = CHUNKS[c]
                    if c == 1 and pr_pending[0] is not None:
                        emit_pr_evict()
                    if s == 0:
                        hi_nb = 15 if c == NCH - 1 else 2 * c
                        while pre_nb[0] <= hi_nb:
                            emit_pre_block(pre_nb[0])
                            pre_nb[0] += 1
                    xq = xpool.tile([128, HK, 8, BC], BF16, tag="xq")
                    if s == 0:
                        # ph == 0: tanh reads Hp directly, no Z-add
                        nc.scalar.activation(
                            out=xq[:, :, 0:tn, :],
                            in_=HPR[:, :, t0 * BC : (t0 + tn) * BC].rearrange(
                                "p h (t b) -> p h t b", b=BC
                            ),
                            func=AF.Tanh,
                        )
                    else:
                        # Z = Hp + ph, one 3D op per h-chunk (2x DVE mode)
                        for hc in range(HK):
                            nc.vector.tensor_tensor(
                                out=xq[:, hc, 0:tn, :],
                                in0=HPR[
                                    :, hc, t0 * BC : (t0 + tn) * BC
                                ].rearrange("p (t b) -> p t b", b=BC),
                                in1=ph_sb[:, hc, :]
                                .unsqueeze(1)
                                .broadcast_to([128, tn, BC]),
                                op=ALU.add,
                            )
                        nc.scalar.activation(
                            out=xq[:, :, 0:tn, :],
                            in_=xq[:, :, 0:tn, :],
                            func=AF.Tanh,
                        )
                    if pend:
                        flush(*pend.pop())
                    e_ps = e_psp.tile([128, 8], F32, tag="e_ps")
                    for tl in range(tn):
                        for hc in range(HK):
                            nc.tensor.matmul(
                                e_ps[:, tl : tl + 1],
                                xq[:, hc, tl, :],
                                WSP[:, hc : hc + 1],
                                start=(hc == 0),
                                stop=(hc == HK - 1),
                            )
                    pend.append((c, e_ps))
                flush(*pend.pop())

                # -- softmax denominator -> rs = 1/sum --
                nc.vector.tensor_reduce(
                    out=RS, in_=SUMS, axis=mybir.AxisListType.X, op=ALU.add
                )
                nc.vector.reciprocal(out=RS, in_=RS)

                # -- ctx evict (normalized, one op) + 4 transposes into one
                # PSUM tile + one bulk copy --
                ctx_sb = ctxp.tile([128, D], BF16, tag="ctx_sb")
                xT = xtp.tile([128, DK, BC], BF16, tag="xT")
                nc.vector.tensor_scalar(
                    out=ctx_sb,
                    in0=ctx_ps,
                    scalar1=RS,
                    scalar2=None,
                    op0=ALU.mult,
                )
                tp = sm_psp.tile([128, 512], BF16, tag="tpsm", name="tpb")
                for md in range(DK):
                    nc.tensor.transpose(
                        tp[:, md * 128 : (md + 1) * 128],
                        ctx_sb[:, md * 128 : (md + 1) * 128],
                        IDBF,
                    )
                nc.vector.tensor_copy(
                    out=xT.rearrange("p k b -> p (k b)"), in_=tp
                )

                # -- gates x-part (ctx) completes each gate group --
                if s == 0:
                    for ng in NGO:
                        g_t[ng] = g_psp.tile(
                            [128, 512], F32, tag=f"gate{ng}", name="g_t"
                        )
                    for ng in NGO:
                        nc.tensor.matmul(
                            g_t[ng],
                            oh_tiles[0],
                            WCAT[:, 4, ng * 512 : (ng + 1) * 512],
                            start=True,
                            stop=False,
                        )
                for ng in NGO:
                    for k in range(DK):
                        nc.tensor.matmul(
                            g_t[ng],
                            xT[:, k, :],
                            WCAT[:, k, ng * 512 : (ng + 1) * 512],
                            start=False,
                            stop=(k == DK - 1),
                        )

                # -- LSTM pointwise; sigmoid via tanh --
                # pointwise scratch aliases a dead xq score buffer
                pwbuf = xpool.tile([128, HK, 8, BC], BF16, tag="xq", name="pwbuf")
                pwflat = pwbuf.rearrange("p h t b -> p (h t b)")
                tifo = pwflat[:, 0 : 3 * 512]
                nc.scalar.activation(
                    out=tifo[:, 512:1024],
                    in_=g_t[1],
                    func=AF.Tanh,
                    scale=0.5,
                )
                p1 = fpool.tile([128, 512], BF16, tag="pw")
                # sigma(f) written in place over tanh(f/2) in tifo
                nc.vector.tensor_scalar(
                    out=tifo[:, 512:1024],
                    in0=tifo[:, 512:1024],
                    scalar1=0.5,
                    scalar2=0.5,
                    op0=ALU.mult,
                    op1=ALU.add,
                )
                nc.vector.tensor_tensor(
                    out=p1, in0=tifo[:, 512:1024], in1=CS, op=ALU.mult
                )
                nc.scalar.activation(
                    out=tifo[:, 0:512],
                    in_=g_t[0],
                    func=AF.Tanh,
                    scale=0.5,
                )
                tg = pwflat[:, 3 * 512 : 4 * 512]
                nc.scalar.activation(
                    out=tg, in_=g_t[3], func=AF.Tanh
                )
                nc.scalar.activation(
                    out=tifo[:, 1024:1536],
                    in_=g_t[2],
                    func=AF.Tanh,
                    scale=0.5,
                )
                p2 = fpool.tile([128, 512], BF16, tag="pw")
                nc.vector.tensor_scalar(
                    out=tifo[:, 0:512],
                    in0=tifo[:, 0:512],
                    scalar1=0.5,
                    scalar2=0.5,
                    op0=ALU.mult,
                    op1=ALU.add,
                )
                nc.vector.tensor_tensor(
                    out=p2, in0=tifo[:, 0:512], in1=tg, op=ALU.mult
                )
                # CS <- p1 + p2 = c_new (sigma form, no 2x factor)
                nc.vector.tensor_tensor(out=CS, in0=p1, in1=p2, op=ALU.add)
                tc2 = tifo[:, 512:1024]  # reuse f-slot (dead after p1)
                nc.scalar.activation(out=tc2, in_=CS, func=AF.Tanh)
                h2x2 = fpool.tile([128, 512], BF16, tag="pw", name="h2x2")
                nc.vector.scalar_tensor_tensor(
                    out=h2x2,
                    in0=tifo[:, 1024:1536],
                    scalar=1.0,
                    in1=tc2,
                    op0=ALU.add,
                    op1=ALU.mult,
                )

                # keep-warm: trivial MMs so the PE HAM window never sees
                # a full idle period during the pointwise tail
                for _ in range(2):
                    kw = e_psp.tile([128, 12], F32, tag="e_ps", name="kw")
                    nc.tensor.matmul(
                        kw[0:1, 0:1], WSP[:, 0:1], WSP[:, 0:1], start=True, stop=True
                    )

                # -- hT = 2*h.T (0.5 folded into Wh/Wg/Whh weights) --
                hT = htpool.tile([128, HK, BC], BF16, tag="ht")
                tp2 = sm_psp.tile([128, 512], BF16, tag="tpsm", name="tpb2")
                for mo in range(HK):
                    nc.tensor.transpose(
                        tp2[:, mo * 128 : (mo + 1) * 128],
                        h2x2[:, mo * 128 : (mo + 1) * 128],
                        IDBF,
                    )
                nc.vector.tensor_copy(
                    out=hT.rearrange("p k b -> p (k b)"), in_=tp2
                )

                # -- ph for next step (critical path): ph = Wh @ h --
                if s + 1 < S:
                    php = sm_psp.tile([128, 512], F32, tag="small", name="php")
                    for mo in range(HK):
                        for k in range(HK):
                            nc.tensor.matmul(
                                php[:, mo * 128 : (mo + 1) * 128],
                                WHT[:, k, mo * 128 : (mo + 1) * 128],
                                hT[:, k, :],
                                start=(k == 0),
                                stop=(k == HK - 1),
                            )
                    ph_sb = phpool.tile([128, HK, BC], BF16, tag="ph")
                    # evict on DVE: idle here, and 2x mode beats ACT Copy
                    nc.vector.tensor_copy(
                        out=ph_sb.rearrange("p k b -> p (k b)"), in_=php
                    )

                # -- probs_s = h @ Wg.T (+bg on evict) -> DRAM --
                pr = sm_psp.tile([128, 512], F32, tag="small")
                for k in range(HK):
                    nc.tensor.matmul(
                        pr[:, 0:C],
                        hT[:, k, :],
                        WGT[:, k, :],
                        start=(k == 0),
                        stop=(k == HK - 1),
                    )
                pr_pending[0] = (pr, s)
            if pr_pending[0] is not None:
                emit_pr_evict()

    nc.compile()
    return nc


def _prep(inputs):
    """Host-side layout prep (casts/transposes/onehots). Returns in_maps."""
    bf = ml_dtypes.bfloat16
    batch_H = np.asarray(inputs["batch_H"], np.float32)
    text = np.asarray(inputs["text"])
    Wi = np.asarray(inputs["Wi"], np.float32)
    Wh = np.asarray(inputs["Wh"], np.float32)
    bh = np.asarray(inputs["bh"], np.float32)
    Ws = np.asarray(inputs["Ws"], np.float32)
    Wih = np.asarray(inputs["Wih"], np.float32)
    Whh = np.asarray(inputs["Whh"], np.float32)
    bih = np.asarray(inputs["bih"], np.float32)
    bhh = np.asarray(inputs["bhh"], np.float32)
    Wg = np.asarray(inputs["Wg"], np.float32)
    bg = np.asarray(inputs["bg"], np.float32)

    bht_full = np.ascontiguousarray(batch_H.transpose(2, 1, 0)).astype(bf)  # [D,T,B]
    bhres_full = batch_H.astype(bf)  # [B,T,D]

    wit = np.ascontiguousarray(Wi.T).reshape(DK, 128, H).astype(bf)
    # hT is stored as 2*h.T (transpose can't scale); fold 0.5 into all
    # weights that consume hT
    wht = np.ascontiguousarray(0.5 * Wh.T).reshape(HK, 128, H).astype(bf)
    wgt = np.ascontiguousarray(0.5 * Wg.T).reshape(HK, 128, C).astype(bf)
    wsp = np.ascontiguousarray(Ws[0].reshape(HK, 128).T).astype(bf)  # [128, HK]
    bhb = np.ascontiguousarray(bh.reshape(HK, 128).T).astype(np.float32)

    # gate permutation: torch (i,f,g,o) -> ours (i,f,o,g)
    perm = np.concatenate(
        [np.arange(0, 1024), np.arange(1536, 2048), np.arange(1024, 1536)]
    )
    Wihp = Wih[perm]
    Whhp = Whh[perm]
    biasp = (bih + bhh)[perm]
    XDIM = 640
    xmat = np.zeros((XDIM, 4 * H), np.float32)
    xmat[0:D] = Wihp[:, 0:D].T
    xmat[D : D + C] = Wihp[:, D : D + C].T
    xmat[D + C] = biasp
    wcat = np.concatenate([xmat, 0.5 * Whhp.T], axis=0)  # [1152, 2048]
    wcat = np.ascontiguousarray(wcat).reshape(9, 128, 4 * H).astype(bf)

    # one-hot (transposed, with constant-1 row at 96) per core
    oht_full = np.zeros((128, S, B), np.float32)
    cb = np.arange(C)
    for s in range(S):
        oht_full[:C, s, :] = (text[:, s][None, :] == cb[:, None]).astype(np.float32)
    oht_full[C, :, :] = 1.0
    oht_full = oht_full.astype(bf)

    prb = np.broadcast_to(bg.reshape(1, C), (128, C)).astype(bf)
    prb = np.ascontiguousarray(prb)
    idbf = np.eye(128, dtype=np.float32).astype(bf)

    in_maps = []
    for c in range(NCORES):
        sl = slice(c * BC, (c + 1) * BC)
        in_maps.append(
            {
                "bht": np.ascontiguousarray(bht_full[:, :, sl]),
                "bhres": np.ascontiguousarray(bhres_full[sl]),
                "wit": wit,
                "wcat": wcat,
                "wht": wht,
                "wgt": wgt,
                "wsp": wsp,
                "bhb": bhb,
                "oht": np.ascontiguousarray(oht_full[:, :, sl]),
                "prb": prb,
                "idbf": idbf,
            }
        )
    return in_maps


def get_nc():
    if "nc" not in _CACHE:
        _CACHE["nc"] = _build()
    return _CACHE["nc"]


def kernel(trace=False, **inputs) -> np.ndarray:
    nc = get_nc()
    in_maps = _prep(inputs)
    res = run_bass_kernel_spmd(
        nc, in_maps, core_ids=list(range(NCORES)), trace=trace
    )
    out = np.concatenate([r["probs"] for r in res.results], axis=0)
    _CACHE["last_results"] = res
    return out
